# revision 19
# baseline (speedup 1.0000x reference)
"""Conv2d(32->32, 3x3, stride 1, pad 1) on X[32,32,224,224] fp32, data-parallel
over 8 NeuronCores (4 images per core).

Per-core algorithm ("column-tiled row-pairs")
---------------------------------------------
Work is split into 56-row half-slices.  X arrives host-padded to 228x226,
host-cast to fp16, and row-rotated: Xr[32*q + c, jd, w] holds padded row
4*jd + q.  A second SBUF copy xr1 (built by two gpsimd SBUF->SBUF
partition-swap DMAs) holds row 4*jq + q + 2, so both row-pair parities
see their four taps in one 128-partition group.

Matmuls are K=128 x M=64 x N=448 (M = 2 output rows x 32 channels,
N = 2 row-pairs x 224 w), issued as COLUMN-TILED pairs: the even-parity
pair streams from xr into PE columns 0..63 (tile_position (0,0)) while
the odd-parity pair streams from xr1 into columns 64..127 ((0,64)).
The two tiles stream through separate XBUSes, so a pair of 448-column
matmuls completes in ~193 ns -- 2x single-matmul throughput -- and every
matmul is K=128 (no K-change drain penalty).  28 dummy matmuls run
before the first X half-tile lands so the PE HAM clock-gate is already
warm.  lhsT[32q+c, s, 32ho+k] = W[k, c, q-ho, s] serves both tiles.

Bias is fused into the PSUM->SBUF eviction (ScalarE/VectorE
alternating), which narrows to fp16 (output max ~90, fp16 rel step
4.9e-4 -- far inside the gate; the host widens back to fp32).  Eviction
is partition-preserving: PSUM partition (pair-parity, ho, k) is exactly
staging partition 32*G + k with G = h%4.  Y is stored to DRAM in
[n, G, k, h//4, w] layout: store descriptors are contiguous 6.3 KB runs
and the (G, k) dims merge to a flat 128 so the runtime splits each store
across all 16 SDMA engines (a non-mergeable outer dim gets only 4; the
natural [k, h, w] layout's 896 B descriptors ran at ~140 GB/s).  The
host un-permutes with one numpy transpose at the end.
"""

import sys

import numpy as np

try:
    import concourse.bass as bass  # noqa: F401
except ImportError:  # pragma: no cover
    sys.path.insert(0, "/opt/trn_rl_repo")

import ml_dtypes
import concourse.mybir as mybir
import concourse.tile as tile
from concourse import bacc
from concourse.bass_utils import run_bass_kernel_spmd

NCORES = 8
NB = 4  # images per core
C = 32
K = 32
H = 224
W = 224
WP = 226  # padded width
NQ = 57  # row-quads in the host-rotated layout (228 padded rows / 4)
RS = 112  # output rows per slice
NSLICE = H // RS
NJD = RS // 4 + 1  # row-quads per slice tile (halo included)
NM = RS // 4  # stored row-quads per slice
F32 = mybir.dt.float32
F16 = mybir.dt.float16
AF = mybir.ActivationFunctionType
_NP16 = np.float16


def conv_body(tc, X, X2, Wt, Bias, Y, Warm):
    nc = tc.nc
    with (
        tc.tile_pool(name="const", bufs=1) as cpool,
        tc.tile_pool(name="xpool", bufs=6) as xpool,
        tc.tile_pool(name="ypool", bufs=6) as ypool,
        tc.tile_pool(name="ppool", bufs=8, space="PSUM") as ppool,
    ):
        # we leads the sync ring (the PE warm-up needs it immediately);
        # bias rides the scalar ring, which spins up while the first X
        # half-tile is in flight
        we_sb = cpool.tile([128, 3, 64], F16)
        nc.sync.dma_start(out=we_sb[:], in_=Wt)
        b_sb = cpool.tile([128, 1], F32)
        nc.scalar.dma_start(out=b_sb[:], in_=Bias)

        # PE warm-up: garbage matmuls on the weight tile while the first
        # X half-tile is still in flight (HAM un-throttles after ~3.4 us
        # of activity; these run in the DMA shadow).
        wpt = ppool.tile([128, 2, 224], F32, name="pt", tag="pt")
        for d in range(28):
            nc.tensor.matmul(
                wpt[0:64, :, 0:64],
                we_sb[:, 0, :],
                we_sb[:, 0:2, :],
                start=(d == 0),
                stop=(d == 27),
            )
        wsb = ypool.tile([128, 8], F32, name="wsb", tag="wsb")
        nc.scalar.activation(wsb[:, :], wpt[:, 0, 0:8], AF.Identity)
        nc.scalar.dma_start(out=Warm, in_=wsb[:, :])

        NH = NM // 2 + 1  # quads per half-slice tile (halo included)
        for n in range(NB):
            for t in range(NSLICE):
                # two 56-row half-slices, each with its own X half-tile,
                # staging tile and store, so the pipeline advances in
                # ~0.9 MB / 1.6 MB steps
                for blk in range(2):
                    j0 = NM * t + 14 * blk
                    xr = xpool.tile([128, NH, WP], F16, name="xr", tag="xr")
                    nc.sync.dma_start(
                        out=xr[:, :, :], in_=X[n, :, :, j0 : j0 + NH, :]
                    )
                    # rotated layout: xr1[32*q1+c, jq] holds padded row
                    # 4*jq + q1 + 2, host-prepared and loaded directly from
                    # DRAM on the scalar HWDGE ring (single-stage, parallel
                    # with the xr load; SWDGE was ~120 GB/s and an
                    # SBUF->SBUF rotation copy added a dependency stage)
                    xr1 = xpool.tile([128, NH, WP], F16, name="xr1", tag="xr1")
                    nc.scalar.dma_start(
                        out=xr1[:, :, :], in_=X2[n, :, :, j0 : j0 + NH, :]
                    )
                    gs = range(7)
                    ysb = ypool.tile([128, NM // 2, 224], F16, name="ysb", tag="ysb")
                    pts = {
                        i: ppool.tile([128, 2, 224], F32, name="pt", tag="pt")
                        for i in gs
                    }
                    for i in gs:
                        for s in range(3):
                            nc.tensor.matmul(
                                pts[i][0:64, :, :],
                                we_sb[:, s, :],
                                xr[:, 2 * i : 2 * i + 2, s : s + 224],
                                start=(s == 0),
                                stop=(s == 2),
                                tile_position=(0, 0),
                            )
                            nc.tensor.matmul(
                                pts[i][64:128, :, :],
                                we_sb[:, s, :],
                                xr1[:, 2 * i : 2 * i + 2, s : s + 224],
                                start=(s == 0),
                                stop=(s == 2),
                                tile_position=(0, 64),
                            )
                        dst = ysb[:, 2 * i : 2 * i + 2, :]
                        if i % 2 == 0:
                            nc.scalar.activation(
                                dst, pts[i][:, :, :], AF.Identity, bias=b_sb[:, :]
                            )
                        else:
                            nc.vector.tensor_scalar_add(
                                dst, pts[i][:, :, :], b_sb[:, :]
                            )
                    nc.scalar.dma_start(
                        out=Y[n, :, :, j0 : j0 + 14, :], in_=ysb[:, :, :]
                    )


def build_nc(nb=NB):
    assert nb == NB
    nc = bacc.Bacc("TRN2", target_bir_lowering=False, debug=False)
    X = nc.dram_tensor("X", [NB, 4, C, NQ, WP], F16, kind="ExternalInput").ap()
    X2 = nc.dram_tensor("X2", [NB, 4, C, NQ, WP], F16, kind="ExternalInput").ap()
    Wt = nc.dram_tensor("Wt", [128, 3, 64], F16, kind="ExternalInput").ap()
    Bias = nc.dram_tensor("bias", [128, 1], F32, kind="ExternalInput").ap()
    Y = nc.dram_tensor("Y", [NB, 4, K, H // 4, 224], F16, kind="ExternalOutput").ap()
    Warm = nc.dram_tensor("warm", [128, 8], F32, kind="ExternalOutput").ap()
    with tile.TileContext(nc) as tc:
        conv_body(tc, X, X2, Wt, Bias, Y, Warm)
    nc.compile()
    return nc


def prep_weights(Wf, b):
    """lhsT[32q+c, s, 32ho+k] = W[k,c,q-ho,s] for ho in {0,1} (zero
    outside 0<=r<3) -- shared by the even (xr) and odd (xr1) pair
    column-tiles."""
    Wf = np.asarray(Wf, np.float32)
    Wt = np.zeros((128, 3, 64), np.float32)
    for q in range(4):
        for ho in range(2):
            r = q - ho
            if 0 <= r <= 2:
                Wt[32 * q : 32 * (q + 1), :, 32 * ho : 32 * (ho + 1)] = Wf[
                    :, :, r, :
                ].transpose(1, 2, 0)
    bias = np.tile(np.asarray(b, np.float32), 4).reshape(128, 1)
    return Wt.astype(_NP16), bias


def pad_input(X):
    """Pad and pre-rotate rows twice: Xr[n, q, c, jd, w] = padded row
    4*jd + q, Xr2[n, q, c, jq, w] = padded row 4*jq + q + 2."""
    X = np.ascontiguousarray(X, np.float32)
    nb = X.shape[0]
    Xp = np.zeros((nb, C, H + 8, WP), _NP16)
    Xp[:, :, 1 : H + 1, 1 : W + 1] = X
    Xr = Xp[:, :, 0 : H + 4].reshape(nb, C, NQ, 4, WP).transpose(0, 3, 1, 2, 4)
    Xr2 = Xp[:, :, 2 : H + 6].reshape(nb, C, NQ, 4, WP).transpose(0, 3, 1, 2, 4)
    return np.ascontiguousarray(Xr), np.ascontiguousarray(Xr2)


_NC = None


def _get_nc():
    global _NC
    if _NC is None:
        _NC = build_nc(NB)
    return _NC


def kernel(X, W, b, _trace=False):
    Xp, Xp2 = pad_input(X)
    Wt, bias = prep_weights(W, b)
    nc = _get_nc()
    in_maps = [
        {
            "X": Xp[NB * c : NB * (c + 1)],
            "X2": Xp2[NB * c : NB * (c + 1)],
            "Wt": Wt,
            "bias": bias,
        }
        for c in range(NCORES)
    ]
    res = run_bass_kernel_spmd(nc, in_maps, list(range(NCORES)), trace=_trace)
    # per-core result: [n, G, k, m, w] fp16 with output row h = 4*m + G
    y2 = np.concatenate([res.results[c]["Y"] for c in range(NCORES)], axis=0)
    out = np.ascontiguousarray(
        y2.transpose(0, 2, 3, 1, 4).astype(np.float32)
    ).reshape(NCORES * NB, 32, 224, 224)
    if _trace:
        return out, res
    return out


# revision 21
# speedup vs baseline: 1.3036x; 1.3036x over previous
"""Conv2d(32->32, 3x3, stride 1, pad 1) on X[32,32,224,224] fp32, data-parallel
over 8 NeuronCores (4 images per core).

Per-core algorithm ("column-tiled row-pairs")
---------------------------------------------
Work is split into 56-row half-slices.  X arrives host-padded to 228x226,
host-cast to fp16, and row-rotated: Xr[32*q + c, jd, w] holds padded row
4*jd + q.  A second SBUF copy xr1 (built by two gpsimd SBUF->SBUF
partition-swap DMAs) holds row 4*jq + q + 2, so both row-pair parities
see their four taps in one 128-partition group.

Matmuls are K=128 x M=64 x N=448 (M = 2 output rows x 32 channels,
N = 2 row-pairs x 224 w), issued as COLUMN-TILED pairs: the even-parity
pair streams from xr into PE columns 0..63 (tile_position (0,0)) while
the odd-parity pair streams from xr1 into columns 64..127 ((0,64)).
The two tiles stream through separate XBUSes, so a pair of 448-column
matmuls completes in ~193 ns -- 2x single-matmul throughput -- and every
matmul is K=128 (no K-change drain penalty).  28 dummy matmuls run
before the first X half-tile lands so the PE HAM clock-gate is already
warm.  lhsT[32q+c, s, 32ho+k] = W[k, c, q-ho, s] serves both tiles.

Bias is fused into the PSUM->SBUF eviction (ScalarE/VectorE
alternating), which narrows to fp16 (output max ~90, fp16 rel step
4.9e-4 -- far inside the gate; the host widens back to fp32).  Eviction
is partition-preserving: PSUM partition (pair-parity, ho, k) is exactly
staging partition 32*G + k with G = h%4.  Y is stored to DRAM in
[n, G, k, h//4, w] layout: store descriptors are contiguous 6.3 KB runs
and the (G, k) dims merge to a flat 128 so the runtime splits each store
across all 16 SDMA engines (a non-mergeable outer dim gets only 4; the
natural [k, h, w] layout's 896 B descriptors ran at ~140 GB/s).  The
host un-permutes with one numpy transpose at the end.
"""

import sys

import numpy as np

try:
    import concourse.bass as bass  # noqa: F401
except ImportError:  # pragma: no cover
    sys.path.insert(0, "/opt/trn_rl_repo")

import ml_dtypes
import concourse.mybir as mybir
import concourse.tile as tile
from concourse import bacc
from concourse.bass_utils import run_bass_kernel_spmd

NCORES = 8
NB = 4  # images per core
C = 32
K = 32
H = 224
W = 224
WP = 226  # padded width
NQ = 57  # row-quads in the host-rotated layout (228 padded rows / 4)
RS = 112  # output rows per slice
NSLICE = H // RS
NJD = RS // 4 + 1  # row-quads per slice tile (halo included)
NM = RS // 4  # stored row-quads per slice
F32 = mybir.dt.float32
F16 = mybir.dt.float16
AF = mybir.ActivationFunctionType
_NP16 = np.float16


def conv_body(tc, X, Wt, Bias, Y, Warm):
    nc = tc.nc
    with (
        tc.tile_pool(name="const", bufs=1) as cpool,
        tc.tile_pool(name="xpool", bufs=6) as xpool,
        tc.tile_pool(name="ypool", bufs=6) as ypool,
        tc.tile_pool(name="ppool", bufs=8, space="PSUM") as ppool,
    ):
        # we leads the sync ring (the PE warm-up needs it immediately);
        # bias rides the scalar ring, which spins up while the first X
        # half-tile is in flight
        we_sb = cpool.tile([128, 3, 64], F16)
        nc.sync.dma_start(out=we_sb[:], in_=Wt)
        b_sb = cpool.tile([128, 1], F32)
        nc.scalar.dma_start(out=b_sb[:], in_=Bias)

        # PE warm-up: garbage matmuls on the weight tile while the first
        # X half-tile is still in flight (HAM un-throttles after ~3.4 us
        # of activity; these run in the DMA shadow).
        wpt = ppool.tile([128, 2, 224], F32, name="pt", tag="pt")
        for d in range(28):
            nc.tensor.matmul(
                wpt[0:64, :, 0:64],
                we_sb[:, 0, :],
                we_sb[:, 0:2, :],
                start=(d == 0),
                stop=(d == 27),
            )
        wsb = ypool.tile([128, 8], F32, name="wsb", tag="wsb")
        nc.scalar.activation(wsb[:, :], wpt[:, 0, 0:8], AF.Identity)
        nc.scalar.dma_start(out=Warm, in_=wsb[:, :])

        NH = NM // 2 + 1  # quads per half-slice tile (halo included)
        for n in range(NB):
            for t in range(NSLICE):
                # two 56-row half-slices, each with its own X half-tile,
                # staging tile and store, so the pipeline advances in
                # ~0.9 MB / 1.6 MB steps
                for blk in range(2):
                    j0 = NM * t + 14 * blk
                    xr = xpool.tile([128, NH, WP], F16, name="xr", tag="xr")
                    nc.sync.dma_start(
                        out=xr[:, :, :], in_=X[n, :, :, j0 : j0 + NH, :]
                    )
                    # rotation copies split across rings: the plain
                    # partition swap rides the fast sync HWDGE ring, the
                    # shifted half stays on gpsimd SWDGE (whose ~120 GB/s
                    # Q7-bound rate throttled the PE when it carried both)
                    xr1 = xpool.tile([128, NH, WP], F16, name="xr1", tag="xr1")
                    nc.sync.dma_start(
                        out=xr1[0:64, :, :], in_=xr[64:128, :, :]
                    )
                    nc.gpsimd.dma_start(
                        out=xr1[64:128, 0 : NH - 1, :], in_=xr[0:64, 1:NH, :]
                    )
                    gs = range(7)
                    ysb = ypool.tile([128, NM // 2, 224], F16, name="ysb", tag="ysb")
                    pts = {
                        i: ppool.tile([128, 2, 224], F32, name="pt", tag="pt")
                        for i in gs
                    }
                    for i in gs:
                        for s in range(3):
                            nc.tensor.matmul(
                                pts[i][0:64, :, :],
                                we_sb[:, s, :],
                                xr[:, 2 * i : 2 * i + 2, s : s + 224],
                                start=(s == 0),
                                stop=(s == 2),
                                tile_position=(0, 0),
                            )
                            nc.tensor.matmul(
                                pts[i][64:128, :, :],
                                we_sb[:, s, :],
                                xr1[:, 2 * i : 2 * i + 2, s : s + 224],
                                start=(s == 0),
                                stop=(s == 2),
                                tile_position=(0, 64),
                            )
                        dst = ysb[:, 2 * i : 2 * i + 2, :]
                        if i % 2 == 0:
                            nc.scalar.activation(
                                dst, pts[i][:, :, :], AF.Identity, bias=b_sb[:, :]
                            )
                        else:
                            nc.vector.tensor_scalar_add(
                                dst, pts[i][:, :, :], b_sb[:, :]
                            )
                    nc.scalar.dma_start(
                        out=Y[n, :, :, j0 : j0 + 14, :], in_=ysb[:, :, :]
                    )


def build_nc(nb=NB):
    assert nb == NB
    nc = bacc.Bacc("TRN2", target_bir_lowering=False, debug=False)
    X = nc.dram_tensor("X", [NB, 4, C, NQ, WP], F16, kind="ExternalInput").ap()
    Wt = nc.dram_tensor("Wt", [128, 3, 64], F16, kind="ExternalInput").ap()
    Bias = nc.dram_tensor("bias", [128, 1], F32, kind="ExternalInput").ap()
    Y = nc.dram_tensor("Y", [NB, 4, K, H // 4, 224], F16, kind="ExternalOutput").ap()
    Warm = nc.dram_tensor("warm", [128, 8], F32, kind="ExternalOutput").ap()
    with tile.TileContext(nc) as tc:
        conv_body(tc, X, Wt, Bias, Y, Warm)
    nc.compile()
    return nc


def prep_weights(Wf, b):
    """lhsT[32q+c, s, 32ho+k] = W[k,c,q-ho,s] for ho in {0,1} (zero
    outside 0<=r<3) -- shared by the even (xr) and odd (xr1) pair
    column-tiles."""
    Wf = np.asarray(Wf, np.float32)
    Wt = np.zeros((128, 3, 64), np.float32)
    for q in range(4):
        for ho in range(2):
            r = q - ho
            if 0 <= r <= 2:
                Wt[32 * q : 32 * (q + 1), :, 32 * ho : 32 * (ho + 1)] = Wf[
                    :, :, r, :
                ].transpose(1, 2, 0)
    bias = np.tile(np.asarray(b, np.float32), 4).reshape(128, 1)
    return Wt.astype(_NP16), bias


def pad_input(X):
    """Pad and pre-rotate rows twice: Xr[n, q, c, jd, w] = padded row
    4*jd + q, Xr2[n, q, c, jq, w] = padded row 4*jq + q + 2."""
    X = np.ascontiguousarray(X, np.float32)
    nb = X.shape[0]
    Xp = np.zeros((nb, C, H + 8, WP), _NP16)
    Xp[:, :, 1 : H + 1, 1 : W + 1] = X
    Xr = Xp[:, :, 0 : H + 4].reshape(nb, C, NQ, 4, WP).transpose(0, 3, 1, 2, 4)
    Xr2 = Xp[:, :, 2 : H + 6].reshape(nb, C, NQ, 4, WP).transpose(0, 3, 1, 2, 4)
    return np.ascontiguousarray(Xr), np.ascontiguousarray(Xr2)


_NC = None


def _get_nc():
    global _NC
    if _NC is None:
        _NC = build_nc(NB)
    return _NC


def kernel(X, W, b, _trace=False):
    Xp, _ = pad_input(X)
    Wt, bias = prep_weights(W, b)
    nc = _get_nc()
    in_maps = [
        {"X": Xp[NB * c : NB * (c + 1)], "Wt": Wt, "bias": bias}
        for c in range(NCORES)
    ]
    res = run_bass_kernel_spmd(nc, in_maps, list(range(NCORES)), trace=_trace)
    # per-core result: [n, G, k, m, w] fp16 with output row h = 4*m + G
    y2 = np.concatenate([res.results[c]["Y"] for c in range(NCORES)], axis=0)
    out = np.ascontiguousarray(
        y2.transpose(0, 2, 3, 1, 4).astype(np.float32)
    ).reshape(NCORES * NB, 32, 224, 224)
    if _trace:
        return out, res
    return out
